# revision 33
# baseline (speedup 1.0000x reference)
"""Trainium2 Bass kernel for nn_CrossAttention (B=2, L=2048, Hd=1024, H=16 heads).

Sharding: 8 cores = data-parallel over B (2) x tensor-parallel over heads (4/core).
Each core computes q/k/v projections for its 4 heads on its batch, causal flash
attention in the S^T orientation, and a partial output projection. The host sums
the 4 partial proj outputs per batch and adds the (folded) biases.

v3 schedule: the attention inner loop is scalar-engine (exp) bound, so
everything else is arranged to hide under it:
  - Inputs DMA in lq-column chunks; a small prefix (k/q chunk 0, v lk-tiles
    0-3) is computed up front so attention block 0 starts ASAP.
  - Attention runs as a flat software pipeline over steps (b, pr, kt): the
    S matmuls + wide exp ACTIVATE of step i+1 are emitted before the PV
    matmuls of step i, keeping the scalar engine saturated.
  - One [128,2,512] psum S-pair tile (2 banks, 2 bufs) per step; ONE exp
    ACTIVATE covers both heads' trimmed columns; causal diag is zeroed by a
    0/1 bf16 multiply on E.
  - Softmax denominator: ones-column folded into the V stationary (M=65);
    psum row 64 accumulates s alongside O^T.
  - Remaining projection work (k/q chunks 1-3, v tiles 4-15, first 12 output
    tiles) is injected as PE filler between pipeline steps, using 2 spare
    psum banks.
  - Per (b, pr): O+s psum is copied to SBUF immediately (releasing the psum
    bank), then 1/s via reciprocal_approx_fast + gpsimd partition_broadcast.

Bias handling (exact): q-bias added on-device per-partition; k-bias is a per-row
constant in logits so softmax cancels it; v-bias and proj bias fold into a single
host-side row-vector add because softmax rows sum to 1.
"""

import os
import numpy as np
import ml_dtypes

os.environ.setdefault("MYCRO_LOCAL_CACHE", "1")

import concourse.bass as bass
import concourse.bacc as bacc
import concourse.tile as tile
from concourse import mybir

BF16 = mybir.dt.bfloat16
F32 = mybir.dt.float32
AF = mybir.ActivationFunctionType
ALU = mybir.AluOpType


class Cfg:
    def __init__(self, L=2048, Hd=1024, D=64, heads_per_core=4):
        self.L = L            # sequence length
        self.Hd = Hd          # model dim (full)
        self.D = D            # head dim
        self.HPC = heads_per_core
        self.DQ = D * heads_per_core          # per-core projected dim (256)
        self.KT = Hd // 128                   # contraction k-tiles for projections
        self.NP = heads_per_core // 2         # head pairs (2)
        self.NLQ = L // 512                   # lq blocks of 512
        self.NLT = L // 128                   # lk tiles of 128
        self.MT = self.DQ // 128              # m-tiles for q/k projections (2)
        self.NB = L // 512                    # lq chunks for projections


def emit_kernel(tc, cfg, io):
    nc = tc.nc
    c = cfg
    L, KT, NP, NLQ, NLT, MT, NB, DQ = c.L, c.KT, c.NP, c.NLQ, c.NLT, c.MT, c.NB, c.DQ

    x_d, y_d, wq_d, wk_d, wv_d, wp_d, bq_d, tri_d, out_d = (
        io["xT"], io["yT"], io["wq"], io["wk"], io["wv"], io["wp"],
        io["bq"], io["tri"], io["out"],
    )

    const_pool = tc.alloc_tile_pool(name="const", bufs=1)
    w_pool = tc.alloc_tile_pool(name="weights", bufs=1)
    xy_pool = tc.alloc_tile_pool(name="xy", bufs=1)
    act_pool = tc.alloc_tile_pool(name="acts", bufs=1)

    # DMA priority order: wk, then y chunk 0 (k-proj), wq + x chunk 0 (q-proj),
    # wv (v-proj), small consts, remaining chunks, wp (needed last).
    wk_sb = w_pool.tile([128, KT * DQ], BF16, name="wk_sb")
    nc.sync.dma_start(wk_sb[:], wk_d[:])

    y_all = xy_pool.tile([128, KT, L], BF16, name="y_all")
    x_all = xy_pool.tile([128, KT, L], BF16, name="x_all")
    for k in range(KT):
        nc.sync.dma_start(y_all[:, k, 0:512], y_d[:, k, 0:512])

    wq_sb = w_pool.tile([128, KT * DQ], BF16, name="wq_sb")
    nc.sync.dma_start(wq_sb[:], wq_d[:])
    for k in range(KT):
        nc.sync.dma_start(x_all[:, k, 0:512], x_d[:, k, 0:512])

    wv_sb = w_pool.tile([128, KT * DQ], BF16, name="wv_sb")
    nc.sync.dma_start(wv_sb[:], wv_d[:])
    tri3_sb = const_pool.tile([128, 3, 128], BF16, name="tri3_sb")
    nc.sync.dma_start(tri3_sb[:], tri_d[:])
    tri_sb = tri3_sb[:, 0:2, :]
    ident = tri3_sb[:, 2, :]
    bq_sb = const_pool.tile([128, MT], F32, name="bq_sb")
    nc.sync.dma_start(bq_sb[:], bq_d[:])

    for nb in range(1, NB):
        sl = slice(nb * 512, (nb + 1) * 512)
        for k in range(KT):
            nc.sync.dma_start(y_all[:, k, sl], y_d[:, k, sl])
        for k in range(KT):
            nc.sync.dma_start(x_all[:, k, sl], x_d[:, k, sl])
    y_sb = [y_all[:, k, :] for k in range(KT)]
    x_sb = [x_all[:, k, :] for k in range(KT)]

    wp_sb = w_pool.tile([128, NP * 1024], BF16, name="wp_sb")
    nc.sync.dma_start(wp_sb[:], wp_d[:])

    # PE warm-up during the DMA wait: ~7us of junk matmuls releases the HAM
    # clock throttle (1.2 -> 2.4 GHz) before the real prefix work arrives,
    # and a dummy exp pulls the ACT table load off the critical path.
    junk = const_pool.tile([128, 512], BF16, name="junk")
    nc.vector.memset(junk[:], 0.0)
    with tc.tile_pool(name="ps_w", bufs=1, space="PSUM") as ps_w:
        wps = ps_w.tile([128, 512], F32, name="wps", tag="w")
        for r in range(12):
            nc.tensor.matmul(wps[:], junk[:, 0:128], junk[:],
                             start=(r == 0), stop=(r == 11))
        wact = const_pool.tile([128, 4], BF16, name="wact")
        nc.scalar.activation(wact[:], wps[:, 0:4], AF.Exp)

    # persistent per-core activations
    kT_sb = [act_pool.tile([128, L], BF16, name=f"kT_sb{p}", tag=f"k{p}") for p in range(NP)]
    qT_sb = [act_pool.tile([128, L], BF16, name=f"qT_sb{p}", tag=f"q{p}") for p in range(NP)]
    # v with a folded ones column per head: [lk 128][lt][head][64 v | 1 one]
    v_sb = act_pool.tile([128, NLT, 4, 65], BF16, name="v_sb")
    nc.vector.memset(v_sb[:, :, :, 64:65], 1.0)
    # vT staging for the flipped v-projection (all 4 lk chunks)
    vT_sb = act_pool.tile([128, 2, 4, 512], BF16, name="vT_sb")
    ot_sb = [act_pool.tile([128, L], BF16, name=f"ot_sb{p}", tag=f"o{p}") for p in range(NP)]

    inv = 1.0 / np.sqrt(c.D)

    # ---------------- helpers for projection chunks ----------------
    def emit_k_chunk(pool, m, nb, tag=None):
        ps = pool.tile([128, 512], F32, name=f"pk{m}{nb}", tag=tag or f"f")
        for j in range(KT):
            nc.tensor.matmul(
                ps[:],
                wk_sb[:, j * DQ + m * 128: j * DQ + (m + 1) * 128],
                y_sb[j][:, nb * 512:(nb + 1) * 512],
                start=(j == 0), stop=(j == KT - 1),
            )
        nc.vector.tensor_copy(kT_sb[m][:, nb * 512:(nb + 1) * 512], ps[:])

    def emit_q_chunk(pool, m, nb, tag=None):
        ps = pool.tile([128, 512], F32, name=f"pq{m}{nb}", tag=tag or f"f")
        for j in range(KT):
            nc.tensor.matmul(
                ps[:],
                wq_sb[:, j * DQ + m * 128: j * DQ + (m + 1) * 128],
                x_sb[j][:, nb * 512:(nb + 1) * 512],
                start=(j == 0), stop=(j == KT - 1),
            )
        nc.vector.tensor_scalar(
            qT_sb[m][:, nb * 512:(nb + 1) * 512], ps[:],
            inv, bq_sb[:, m:m + 1], ALU.mult, ALU.add,
        )

    def emit_v_flip_g(pool, pr, ch, tag=None):
        # vT chunk: wv-pair stationary, y chunk moving (full-width streams)
        ps = pool.tile([128, 512], F32, name="pvf", tag=tag or "f")
        for j in range(KT):
            nc.tensor.matmul(
                ps[:],
                wv_sb[:, j * DQ + pr * 128: j * DQ + (pr + 1) * 128],
                y_sb[j][:, ch * 512:(ch + 1) * 512],
                start=(j == 0), stop=(j == KT - 1),
            )
        nc.vector.tensor_copy(vT_sb[:, pr, ch, :], ps[:])

    def emit_v_trans_g(pool, pr, ch, tag=None):
        # transpose 4 lk tiles of this pair's vT chunk back to v layout
        ps = pool.tile([128, 4, 2, 64], BF16, name="pvt", tag=tag or "f")
        for i in range(4):
            nc.tensor.transpose(
                ps[:, i, :, :],
                vT_sb[:, pr, ch, i * 128:(i + 1) * 128],
                ident,
            )
        nc.vector.tensor_copy(
            v_sb[:, 4 * ch:4 * ch + 4, 2 * pr:2 * pr + 2, 0:64], ps[:])

    # ---------------- Phase A prefix: just enough for block 0 ----------------
    with tc.tile_pool(name="ps_pre", bufs=1, space="PSUM") as ps_pre:
        for m in range(MT):
            emit_k_chunk(ps_pre, m, 0, tag=f"k{m}")
        for m in range(MT):
            emit_q_chunk(ps_pre, m, 0, tag=f"q{m}")
        with tc.tile_pool(name="ps_prev", bufs=2, space="PSUM") as ps_prev:
            for pr in range(NP):
                emit_v_flip_g(ps_prev, pr, 0, tag="v")
            for pr in range(NP):
                emit_v_trans_g(ps_prev, pr, 0, tag="v")

    # ---------------- Phase B: attention (flat software pipeline) -----------
    e_pool = tc.alloc_tile_pool(name="etiles", bufs=4)
    os_pool = tc.alloc_tile_pool(name="osb", bufs=2)
    rs_pool = tc.alloc_tile_pool(name="recip", bufs=2)
    bc_pool = tc.alloc_tile_pool(name="bcast", bufs=2)
    out_pool = tc.alloc_tile_pool(name="outs", bufs=3)

    steps = [(b, pr, kt) for b in range(NLQ) for pr in range(NP)
             for kt in range(4 * (b + 1))]

    with (
        tc.tile_pool(name="ps_s", bufs=2, space="PSUM") as ps_s,
        tc.tile_pool(name="ps_o", bufs=1, space="PSUM") as ps_o,
        tc.tile_pool(name="ps_f", bufs=2, space="PSUM") as ps_f,
    ):
        # filler units: closures emitting ~1-2us of PE work each, in dep order
        fillers = []
        for nb in range(1, NB):
            for m in range(MT):
                fillers.append(lambda m=m, nb=nb: emit_k_chunk(ps_f, m, nb))
                fillers.append(lambda m=m, nb=nb: emit_q_chunk(ps_f, m, nb))
            for pr in range(NP):
                fillers.append(lambda pr=pr, nb=nb: emit_v_flip_g(ps_f, pr, nb))
            for pr in range(NP):
                fillers.append(lambda pr=pr, nb=nb: emit_v_trans_g(ps_f, pr, nb))

        def emit_c_tile(t, tail=False):
            pa = ps_f.tile([128, 512], F32, name="pca", tag="f")
            pb = ps_f.tile([128, 512], F32, name="pcb", tag="f")
            for pr in range(NP):
                lhsT = ot_sb[pr][:, t * 128:(t + 1) * 128]
                nc.tensor.matmul(
                    pa[:], lhsT, wp_sb[:, pr * 1024: pr * 1024 + 512],
                    start=(pr == 0), stop=(pr == NP - 1))
                nc.tensor.matmul(
                    pb[:], lhsT, wp_sb[:, pr * 1024 + 512: pr * 1024 + 1024],
                    start=(pr == 0), stop=(pr == NP - 1))
            out_t = out_pool.tile([128, 2, 512], BF16, tag="out_t")
            nc.vector.tensor_copy(out_t[:, 0, :], pa[:])
            if tail:
                # scalar engine is idle after the last exp; split the casts
                nc.scalar.activation(out_t[:, 1, :], pb[:], AF.Copy)
            else:
                nc.vector.tensor_copy(out_t[:, 1, :], pb[:])
            nc.sync.dma_start(out_d[t], out_t[:])

        o_tiles = {}

        def emit_s_act(i):
            b, pr, kt = steps[i]
            p = kt - 4 * b
            c0 = max(p, 0) * 128
            q_sl = slice(b * 512 + c0, (b + 1) * 512)
            sp = ps_s.tile([128, 2, 512], F32, tag="sp")
            nc.tensor.matmul(
                sp[:, 0, c0:512],
                kT_sb[pr][0:64, kt * 128:(kt + 1) * 128],
                qT_sb[pr][0:64, q_sl],
                start=True, stop=True,
            )
            nc.tensor.matmul(
                sp[:, 1, c0:512],
                kT_sb[pr][64:128, kt * 128:(kt + 1) * 128],
                qT_sb[pr][64:128, q_sl],
                start=True, stop=True,
            )
            e = e_pool.tile([128, 2, 512], BF16, tag="e")
            nc.scalar.activation(e[:, :, c0:512], sp[:, :, c0:512], AF.Exp)
            if p >= 0:
                nc.vector.tensor_tensor(
                    e[:, :, c0:c0 + 128], e[:, :, c0:c0 + 128],
                    tri_sb[:], ALU.mult,
                )
            return e

        def emit_pv(i, e):
            b, pr, kt = steps[i]
            nkt = 4 * (b + 1)
            p = kt - 4 * b
            c0 = max(p, 0) * 128
            if kt == 0:
                o_tiles[0] = ps_o.tile([65, 2, 512], F32, name="o_pair", tag="o")
            for h in range(2):
                nc.tensor.matmul(
                    o_tiles[0][:, h, c0:512],
                    v_sb[:, kt, 2 * pr + h, :],
                    e[:, h, c0:512],
                    start=(kt == 0), stop=(kt == nkt - 1),
                )

        def emit_normalize(b, pr):
            # copy O+s to SBUF in one shot (releases the psum o banks),
            # then 1/s -> broadcast -> scale into bf16 ot.
            o_c = os_pool.tile([65, 2, 512], F32, name="o_c", tag="oc")
            nc.vector.tensor_copy(o_c[:], o_tiles[0][:])
            s_pair = rs_pool.tile([1, 2, 512], F32, name="s_pair", tag="sp")
            nc.vector.tensor_copy(s_pair[:], o_c[64:65, :, :])
            rs = rs_pool.tile([1, 2, 512], F32, tag="rs")
            nc.vector.reciprocal_approx_fast(rs[:], s_pair[:])
            bc = bc_pool.tile([64, 2, 512], F32, tag="bc")
            nc.gpsimd.partition_broadcast(bc[:], rs[:], channels=64)
            for h in range(2):
                nc.vector.tensor_tensor(
                    ot_sb[pr][h * 64:(h + 1) * 64, b * 512:(b + 1) * 512],
                    o_c[0:64, h, :], bc[:, h, :], ALU.mult,
                )

        # pipeline: S/ACT one step ahead of PV
        e_cur = emit_s_act(0)
        for i, (b, pr, kt) in enumerate(steps):
            e_next = emit_s_act(i + 1) if i + 1 < len(steps) else None
            emit_pv(i, e_cur)
            e_cur = e_next
            if kt == 4 * (b + 1) - 1:
                emit_normalize(b, pr)
                if pr == NP - 1 and b < NLQ - 1:
                    # ot block b complete for both pairs: out tiles 4b..4b+3
                    for t in range(4 * b, 4 * b + 4):
                        fillers.append(lambda t=t: emit_c_tile(t))
            if fillers:
                fillers.pop(0)()

        while fillers:
            fillers.pop(0)()

        # ---------------- Phase C tail: last out tiles ----------------
        for t in range(12, NLT):
            emit_c_tile(t, tail=True)

    # release in reverse allocation (stack) order
    out_pool.release()
    bc_pool.release()
    rs_pool.release()
    os_pool.release()
    e_pool.release()
    act_pool.release()
    xy_pool.release()
    w_pool.release()
    const_pool.release()


def build_nc(cfg):
    """Build the Bass program for one core (identical across cores)."""
    c = cfg
    nc = bacc.Bacc("TRN2", target_bir_lowering=False, debug=False)
    io = {
        "xT": nc.dram_tensor("xT", [128, c.KT, c.L], BF16, kind="ExternalInput").ap(),
        "yT": nc.dram_tensor("yT", [128, c.KT, c.L], BF16, kind="ExternalInput").ap(),
        "wq": nc.dram_tensor("wq", [128, c.KT * c.DQ], BF16, kind="ExternalInput").ap(),
        "wk": nc.dram_tensor("wk", [128, c.KT * c.DQ], BF16, kind="ExternalInput").ap(),
        "wv": nc.dram_tensor("wv", [128, c.KT * c.DQ], BF16, kind="ExternalInput").ap(),
        "wp": nc.dram_tensor("wp", [128, c.NP * 1024], BF16, kind="ExternalInput").ap(),
        "bq": nc.dram_tensor("bq", [128, c.MT], F32, kind="ExternalInput").ap(),
        "tri": nc.dram_tensor("tri", [128, 3, 128], BF16, kind="ExternalInput").ap(),
        "out": nc.dram_tensor("out", [c.NLT, 128, 2, 512], BF16, kind="ExternalOutput").ap(),
    }
    with tile.TileContext(nc) as tc:
        emit_kernel(tc, c, io)
    nc.compile()
    return nc


def _bf(a):
    return np.ascontiguousarray(a).astype(ml_dtypes.bfloat16)


def make_in_map(cfg, x_b, y_b, Wq_c, Wq_b_c, Wk_c, Wv_c):
    """Per-core input map. x_b/y_b: (L, Hd) fp32 for this core's batch.
    Wq_c/Wk_c/Wv_c: (Hd, DQ) column slices. Wq_b_c: (DQ,) bias slice."""
    c = cfg
    xT = np.ascontiguousarray(x_b.T.reshape(c.KT, 128, c.L).transpose(1, 0, 2))
    yT = np.ascontiguousarray(y_b.T.reshape(c.KT, 128, c.L).transpose(1, 0, 2))
    # weight slabs: [Hd, DQ] -> [KT, 128, DQ] -> [128, KT*DQ]
    def slab(w):
        return np.ascontiguousarray(
            w.reshape(c.KT, 128, c.DQ).transpose(1, 0, 2).reshape(128, c.KT * c.DQ))
    bq = (Wq_b_c.astype(np.float32) / np.sqrt(c.D)).reshape(c.MT, 128).T
    r = np.arange(128)
    tri01 = np.where(r[:, None] <= r[None, :], 1.0, 0.0).astype(np.float32)
    ident = np.eye(128, dtype=np.float32)
    tri2 = np.stack([tri01, tri01, ident], axis=1)  # [128, 3, 128]
    return {
        "xT": _bf(xT), "yT": _bf(yT),
        "wq": _bf(slab(Wq_c)), "wk": _bf(slab(Wk_c)), "wv": _bf(slab(Wv_c)),
        "bq": np.ascontiguousarray(bq).astype(np.float32),
        "tri": _bf(tri2),
    }


def _numpy_reference(x, y, mask, Wq_w, Wq_b, Wkv_w, Wkv_b, proj_w, proj_b):
    """Exact fallback (only used if the padding mask is nonzero)."""
    B, L, Hd = x.shape
    H = 16
    D = Hd // H
    q = (x.reshape(-1, Hd) @ Wq_w + Wq_b).reshape(B, L, H, D)
    kv = (y.reshape(-1, Hd) @ Wkv_w + Wkv_b).reshape(B, L, 2, H, D)
    k, v = kv[:, :, 0], kv[:, :, 1]
    out = np.zeros((B, L, Hd), np.float32)
    causal = np.triu(np.ones((L, L), bool), 1)
    for b in range(B):
        comb = causal | mask[b][None, :]
        for h in range(H):
            S = (q[b, :, h] @ k[b, :, h].T) / np.sqrt(D)
            S = np.where(comb, -np.inf, S)
            S = S - S.max(axis=1, keepdims=True)
            E = np.exp(S)
            P = E / E.sum(axis=1, keepdims=True)
            out[b, :, h * D:(h + 1) * D] = P @ v[b, :, h]
    return (out.reshape(-1, Hd) @ proj_w + proj_b).reshape(B, L, Hd).astype(np.float32)


_NC_CACHE = {}


def _get_nc(cfg):
    key = (cfg.L, cfg.Hd, cfg.D, cfg.HPC)
    if key not in _NC_CACHE:
        _NC_CACHE[key] = build_nc(cfg)
    return _NC_CACHE[key]


def kernel(x, y, mask, Wq_w, Wq_b, Wkv_w, Wkv_b, proj_w, proj_b, **run_kwargs):
    x = np.asarray(x, np.float32)
    y = np.asarray(y, np.float32)
    mask = np.asarray(mask)
    Wq_w = np.asarray(Wq_w, np.float32)
    Wq_b = np.asarray(Wq_b, np.float32)
    Wkv_w = np.asarray(Wkv_w, np.float32)
    Wkv_b = np.asarray(Wkv_b, np.float32)
    proj_w = np.asarray(proj_w, np.float32)
    proj_b = np.asarray(proj_b, np.float32)

    if mask.any():
        return _numpy_reference(x, y, mask, Wq_w, Wq_b, Wkv_w, Wkv_b, proj_w, proj_b)

    B, L, Hd = x.shape
    H = 16
    D = Hd // H
    cfg = Cfg(L=L, Hd=Hd, D=D, heads_per_core=4)
    n_cores = 8
    tp = n_cores // B  # 4 tensor-parallel cores per batch

    # kv weight split: (Hd, 2, H, D)
    Wkv_r = Wkv_w.reshape(Hd, 2, H, D)
    Wkv_b_r = Wkv_b.reshape(2, H, D)

    nc = _get_nc(cfg)

    in_maps = []
    for core in range(n_cores):
        b = core // tp
        h0 = (core % tp) * cfg.HPC
        cols = slice(h0 * D, (h0 + cfg.HPC) * D)
        Wq_c = Wq_w[:, cols]
        Wq_b_c = Wq_b[cols]
        Wk_c = Wkv_r[:, 0, h0:h0 + cfg.HPC].reshape(Hd, cfg.DQ)
        Wv_c = Wkv_r[:, 1, h0:h0 + cfg.HPC].reshape(Hd, cfg.DQ)
        im = make_in_map(cfg, x[b], y[b], Wq_c, Wq_b_c, Wk_c, Wv_c)
        # per-core proj rows slab: (DQ, 1024) -> [NP, 128, 1024] -> [128, NP*1024]
        Wp_c = proj_w[cols, :]
        im["wp"] = _bf(Wp_c.reshape(cfg.NP, 128, Hd).transpose(1, 0, 2).reshape(128, cfg.NP * Hd))
        in_maps.append(im)

    from concourse.bass_utils import run_bass_kernel_spmd
    res = run_bass_kernel_spmd(nc, in_maps, core_ids=list(range(n_cores)), **run_kwargs)

    # host-side unshard: sum partials per batch, add folded biases
    # (k-bias cancels in softmax; v-bias @ proj_w + proj_b is a constant row)
    bias_row = proj_b + Wkv_b_r[1].reshape(Hd) @ proj_w
    out = np.zeros((B, L, Hd), np.float32)
    for core in range(n_cores):
        b = core // tp
        out[b] += res.results[core]["out"].astype(np.float32).reshape(L, Hd)
    out += bias_row[None, None, :]
    if getattr(kernel, "_return_results", False):
        kernel._last_results = res
    return out
